# revision 57
# baseline (speedup 1.0000x reference)
"""DinoV2 attention (B=8, S=1370, D=1024, H=16, Dh=64) on 8 trn2 NeuronCores.

Sharding: data parallel over batch — core b computes batch element b end to
end; weights are replicated; no collectives.

Per-core design (all matmul operands bf16 — full PE rate at any free size,
half the DMA bytes; PSUM accumulation stays fp32):

  xT    = x[b].T (host-transposed, bf16)       [D, S]
  per head pair hp (feature block hp*128 .. hp*128+127):
    QT/KT = Wq_hp.T @ xT (+b)                  [128, S]  bf16, lhsT=W tiles
    V_hp  = xT.T @ Wv_hp (+bv), ones col       [S, 2, 65] into v_sb (bf16)
    S_h   = K_h @ Q_h.T                        [S_k, S_q] both heads into one
                                               2-bank PSUM tile [128, 2, 512]
    E     = exp(S/8)                           one ScalarE op for BOTH heads,
                                               bf16 out -> eAB [128,11,2,512]
    O_q   = E_h.T @ [V_h | 1]                  [q, 65] per 128-q subtile; the
                                               lhsT/rhs swap puts q on
                                               partitions so the softmax
                                               denominator Z (col 64) is a
                                               per-partition scalar: recip +
                                               tensor_scalar_mul, no DRAM
                                               partition-broadcast bounce.
    OT    = PE-transpose(O_q pair tile)        [128(d), q] -> ot_sb
  out = OT.T @ Wo + bo                         [S, D]  fp32 out

The emission order (proj(hp) -> V(hp) -> attn(hp)) lets the Tile list
scheduler fill ScalarE-bound attention windows of pair hp with the
projection/V matmuls of pair hp+1, keeping PE busy.
"""

import numpy as np
import ml_dtypes
from contextlib import ExitStack

import concourse.bass as bass
import concourse.mybir as mybir
import concourse.tile as tile
from concourse.bass_utils import run_bass_kernel_spmd
from concourse.masks import make_identity

B = 8
S = 1370
D = 1024
H = 16
DH = 64
P = 128
KT = D // P              # 8 contraction tiles over D
NPAIR = H // 2           # 8 head pairs
NST = (S + P - 1) // P   # 11 s-tiles (last one is 90 rows)
FP = mybir.dt.float32
BF = mybir.dt.bfloat16
AF = mybir.ActivationFunctionType

ST_SIZES = [min(P, S - i * P) for i in range(NST)]
Q_CHUNKS = [(0, 512), (512, 512), (1024, S - 1024)]   # free-dim chunks of S
SCALE = 1.0 / np.sqrt(DH)


def _legalize_syncs(nc):
    """Move excess sem waits onto injected NoOps.

    This walrus build encodes at most one wait (plus one update) per TPB
    instruction; Tile emits several. Engines execute their streams in
    order and the Tile schedule is a topological order of the dependency
    DAG, so hoisting waits onto preceding same-engine NoOps preserves
    progress (anything scheduled earlier can still complete) and
    correctness (the instruction still starts only after all its waits).
    """
    nid = 0
    for f in nc.m.functions:
        for blk in f.blocks:
            out = []
            for inst in blk.instructions:
                si = inst.sync_info
                if si is not None:
                    waits = list(si.on_wait)
                    ups = list(si.on_update)
                    if len(waits) > 1:
                        for w in waits[:-1]:
                            nop = mybir.InstNoOp(
                                name=f"I-syncfix-{nid}",
                                engine=inst.engine, ins=[], outs=[],
                                sync_info=mybir.SyncInfo(on_wait=[w],
                                                         on_update=[]))
                            nid += 1
                            nc.register_instruction(nop)
                            out.append(nop)
                        inst.sync_info = mybir.SyncInfo(on_wait=waits[-1:],
                                                        on_update=ups)
                out.append(inst)
            blk.instructions = out


def _sub_tiles(cw):
    """128-wide q subtiles of a q chunk."""
    subs = []
    off = 0
    while off < cw:
        subs.append((off, min(P, cw - off)))
        off += P
    return subs


def build_nc(repeat=1):
    nc = bass.Bass()
    xT = nc.declare_dram_parameter("xT", [D, S], BF, isOutput=False)
    Wq = nc.declare_dram_parameter("Wq", [D, D], BF, isOutput=False)
    Wk = nc.declare_dram_parameter("Wk", [D, D], BF, isOutput=False)
    Wv = nc.declare_dram_parameter("Wv", [D, D], BF, isOutput=False)
    Wo = nc.declare_dram_parameter("Wo", [D, D], BF, isOutput=False)
    bq = nc.declare_dram_parameter("bq", [D], FP, isOutput=False)
    bk = nc.declare_dram_parameter("bk", [D], FP, isOutput=False)
    bv = nc.declare_dram_parameter("bv", [D], FP, isOutput=False)
    bo = nc.declare_dram_parameter("bo", [D], FP, isOutput=False)
    out = nc.declare_dram_parameter("out", [S, D], FP, isOutput=True)

    def bcast128(handle):
        # [D] dram vector replicated across 128 partitions
        a = handle[:]
        return bass.AP(tensor=a.tensor, offset=a.offset,
                       ap=[[0, P], *a.ap])

    wq_r = Wq[:].rearrange("(kt p) c -> p kt c", p=P)
    wk_r = Wk[:].rearrange("(kt p) c -> p kt c", p=P)
    wv_r = Wv[:].rearrange("(kt p) c -> p kt c", p=P)

    with ExitStack() as ctx:
        tc = ctx.enter_context(tile.TileContext(nc))
        const = ctx.enter_context(tc.tile_pool(name="const", bufs=1))
        persist = ctx.enter_context(tc.tile_pool(name="persist", bufs=1))
        psum = ctx.enter_context(tc.tile_pool(name="psum", bufs=1,
                                              space="PSUM"))

        # biases: per-partition layout for QT/KT (feature on partitions),
        # partition-broadcast layout for V (feature on free dim); DMAs are
        # emitted after the first xT/weight stream (load_consts) since the
        # first consumers run ~10us in
        bq_sb = const.tile([P, NPAIR], FP)
        bk_sb = const.tile([P, NPAIR], FP)
        bv_bc = const.tile([P, D], FP)
        ident = const.tile([P, P], BF)
        make_identity(nc, ident)


        def load_consts():
            nc.sync.dma_start(out=bq_sb,
                              in_=bq[:].rearrange("(kt p) -> p kt", p=P))
            nc.sync.dma_start(out=bk_sb,
                              in_=bk[:].rearrange("(kt p) -> p kt", p=P))
            nc.sync.dma_start(out=bv_bc, in_=bcast128(bv))

        # persistent: V (with fused ones column per head) and transposed O
        v_sb = persist.tile([P, NST, H, DH + 1], BF)
        ot_sb = persist.tile([P, NPAIR, S], BF)
        for st in range(NST):
            nc.gpsimd.memset(v_sb[:, st, :, DH:DH + 1], 1.0)

        wop = ctx.enter_context(tc.tile_pool(name="wop", bufs=1))

        for _rep in range(repeat):
            with tc.tile_pool(name="xp", bufs=1) as xp, \
                 tc.tile_pool(name="wp", bufs=2) as wp, \
                 tc.tile_pool(name="qkp", bufs=2) as qkp, \
                 tc.tile_pool(name="ep", bufs=3) as ep, \
                 tc.tile_pool(name="op", bufs=2) as op:
                wo_sb = wop.tile([P, KT, D], BF, tag="wo")
                bo_bc = wop.tile([P, D], FP, tag="bobc")
                xT_sb = xp.tile([P, KT, S], BF)
                xT_r = xT[:].rearrange("(kt p) s -> p kt s", p=P)

                def load_pair_weights(hp):
                    wq_t = wp.tile([P, KT, P], BF, tag="wq")
                    wk_t = wp.tile([P, KT, P], BF, tag="wk")
                    wv_t = wp.tile([P, KT, P], BF, tag="wv")
                    c0 = hp * P
                    nc.sync.dma_start(out=wq_t, in_=wq_r[:, :, c0:c0 + P])
                    nc.sync.dma_start(out=wk_t, in_=wk_r[:, :, c0:c0 + P])
                    nc.sync.dma_start(out=wv_t, in_=wv_r[:, :, c0:c0 + P])
                    return wq_t, wk_t, wv_t

                # pair-0 weights interleaved into the head of the xT stream
                # (split in halves) so the kt-major warmup chains start on
                # the first chunks as early as possible
                wq_t0 = wp.tile([P, KT, P], BF, tag="wq")
                wk_t0 = wp.tile([P, KT, P], BF, tag="wk")
                wv_t0 = wp.tile([P, KT, P], BF, tag="wv")
                nc.sync.dma_start(out=wq_t0[:, 0:1, :], in_=wq_r[:, 0:1, 0:P])
                nc.sync.dma_start(out=xT_sb[:, 0, :], in_=xT_r[:, 0, :])
                nc.sync.dma_start(out=wk_t0[:, 0:1, :], in_=wk_r[:, 0:1, 0:P])
                nc.sync.dma_start(out=xT_sb[:, 1, :], in_=xT_r[:, 1, :])
                nc.sync.dma_start(out=wq_t0[:, 1:8, :], in_=wq_r[:, 1:8, 0:P])
                nc.sync.dma_start(out=wk_t0[:, 1:8, :], in_=wk_r[:, 1:8, 0:P])
                for kt in range(2, KT):
                    nc.sync.dma_start(out=xT_sb[:, kt, :], in_=xT_r[:, kt, :])
                load_consts()
                nc.sync.dma_start(out=wv_t0, in_=wv_r[:, :, 0:P])
                nxt = (wq_t0, wk_t0, wv_t0)
                for hp in range(NPAIR):
                    wq_t, wk_t, wv_t = nxt
                    if hp + 1 < NPAIR:
                        nxt = load_pair_weights(hp + 1)
                    if hp == 6:
                        # prefetch the out-projection weights mid-stream so
                        # phase C's chains fill the pair-7 attention tail
                        wo_r = Wo[:].rearrange("(kt p) c -> p kt c", p=P)
                        for g in range(2):
                            nc.sync.dma_start(
                                out=wo_sb[:, :, g * 512:(g + 1) * 512],
                                in_=wo_r[:, :, g * 512:(g + 1) * 512])
                        nc.sync.dma_start(out=bo_bc, in_=bcast128(bo))

                    # ---- Q/K projections for this pair: QT/KT [128, S]
                    qt_sb = qkp.tile([P, S], BF, tag="qt")
                    kt_sb = qkp.tile([P, S], BF, tag="kt")
                    if hp == 0:
                        # kt-major warmup: all 6 accumulation chains in
                        # parallel across the idle attention PSUM banks, so
                        # PE keeps pace with the incoming xT DMA stream
                        s2t = psum.tile([P, 2, 512], FP, tag="s2", bufs=2)
                        ppt = psum.tile([P, 512], FP, tag="pp", bufs=2)
                        ppt2 = psum.tile([P, 512], FP, tag="pp", bufs=2)
                        avt = psum.tile([P, 2, 512], FP, tag="av", bufs=1)
                        chains = [(wq_t, s2t[:, 0, :], 0, 512),
                                  (wq_t, s2t[:, 1, :], 512, 512),
                                  (wq_t, ppt[:, 0:346], 1024, 346),
                                  (wk_t, avt[:, 0, :], 0, 512),
                                  (wk_t, avt[:, 1, :], 512, 512),
                                  (wk_t, ppt2[:, 0:346], 1024, 346)]
                        for kt in range(KT):
                            st_, sp_ = kt == 0, kt == KT - 1
                            for (wt, pt, q0, cw) in chains:
                                nc.tensor.matmul(
                                    pt, lhsT=wt[:, kt, :],
                                    rhs=xT_sb[:, kt, q0:q0 + cw],
                                    start=st_, stop=sp_)
                        # evictions ordered by score consumption (k tiles
                        # sweep all of S, so kt chunks gate exps first);
                        # alternates go to the idle GpSimd engine
                        nc.vector.tensor_scalar_add(
                            kt_sb[:, 0:512], avt[:, 0, :], bk_sb[:, 0:1])
                        nc.scalar.activation(
                            qt_sb[:, 0:512], s2t[:, 0, :], AF.Identity,
                            bias=bq_sb[:, 0:1])
                        nc.vector.tensor_scalar_add(
                            kt_sb[:, 512:1024], avt[:, 1, :], bk_sb[:, 0:1])
                        nc.scalar.activation(
                            qt_sb[:, 512:1024], s2t[:, 1, :], AF.Identity,
                            bias=bq_sb[:, 0:1])
                        nc.vector.tensor_scalar_add(
                            qt_sb[:, 1024:S], ppt[:, 0:346], bq_sb[:, 0:1])
                        nc.vector.tensor_scalar_add(
                            kt_sb[:, 1024:S], ppt2[:, 0:346], bk_sb[:, 0:1])
                    else:
                        for (q0, cw) in Q_CHUNKS:
                            qps = psum.tile([P, 512], FP, tag="pp", bufs=2)
                            for kt in range(KT):
                                nc.tensor.matmul(
                                    qps[:, :cw], lhsT=wq_t[:, kt, :],
                                    rhs=xT_sb[:, kt, q0:q0 + cw],
                                    start=(kt == 0), stop=(kt == KT - 1))
                            with tc.high_priority(offset=2000):
                                nc.vector.tensor_scalar_add(
                                    qt_sb[:, q0:q0 + cw], qps[:, :cw],
                                    bq_sb[:, hp:hp + 1])
                            kps = psum.tile([P, 512], FP, tag="pp", bufs=2)
                            for kt in range(KT):
                                nc.tensor.matmul(
                                    kps[:, :cw], lhsT=wk_t[:, kt, :],
                                    rhs=xT_sb[:, kt, q0:q0 + cw],
                                    start=(kt == 0), stop=(kt == KT - 1))
                            with tc.high_priority(offset=2000):
                                nc.vector.tensor_scalar_add(
                                    kt_sb[:, q0:q0 + cw], kps[:, :cw],
                                    bk_sb[:, hp:hp + 1])

                    # ---- V projection for this pair's two heads
                    for st in range(NST):
                        sw = ST_SIZES[st]
                        vps = psum.tile([P, 512], FP, tag="pp", bufs=2)
                        for kt in range(KT):
                            nc.tensor.matmul(
                                vps[:sw, :P],
                                lhsT=xT_sb[:, kt, st * P:st * P + sw],
                                rhs=wv_t[:, kt, :],
                                start=(kt == 0), stop=(kt == KT - 1))
                        nc.vector.tensor_add(
                            v_sb[:sw, st, 2 * hp:2 * hp + 2, 0:DH],
                            vps[:sw, :P].rearrange("p (h d) -> p h d", h=2),
                            bv_bc[:sw, hp * P:(hp + 1) * P].rearrange(
                                "p (h d) -> p h d", h=2))

                    # ---- attention for heads (2*hp, 2*hp+1)
                    for (q0, cw) in Q_CHUNKS:
                        # scores + exp, both heads fused per k-tile
                        eAB = ep.tile([P, NST, 2, 512], BF, tag="e")
                        for ks in range(NST):
                            k0, kw = ks * P, ST_SIZES[ks]
                            s2 = psum.tile([P, 2, 512], FP, tag="s2", bufs=2)
                            # scores feed the bottleneck ScalarE exp stream:
                            # let them win ready-heap ties against AV /
                            # transpose / projection work
                            with tc.high_priority(offset=6000):
                                nc.tensor.matmul(
                                    s2[:kw, 0, :cw],
                                    lhsT=kt_sb[0:DH, k0:k0 + kw],
                                    rhs=qt_sb[0:DH, q0:q0 + cw],
                                    start=True, stop=True,
                                    tile_position=(0, 0))
                                nc.tensor.matmul(
                                    s2[:kw, 1, :cw],
                                    lhsT=kt_sb[DH:P, k0:k0 + kw],
                                    rhs=qt_sb[DH:P, q0:q0 + cw],
                                    start=True, stop=True,
                                    tile_position=(64, 0))
                            nc.scalar.activation(
                                eAB[:kw, ks, :, 0:cw], s2[:kw, :, 0:cw],
                                AF.Exp, scale=float(SCALE))

                        # O = E.T @ [V | 1] per 128-q subtile; q lands on
                        # partitions so Z is a per-partition scalar
                        # O = E.T @ [V | 1]: bank 0 head A, bank 1 head B
                        # (accumulation groups are bank-granular)
                        for (qoff, sub) in _sub_tiles(cw):
                            av = psum.tile([P, 2, 512], FP, tag="av", bufs=1)
                            for ks in range(NST):
                                kw = ST_SIZES[ks]
                                nc.tensor.matmul(
                                    av[:sub, 0, 0:DH + 1],
                                    lhsT=eAB[0:kw, ks, 0, qoff:qoff + sub],
                                    rhs=v_sb[0:kw, ks, 2 * hp, :],
                                    start=(ks == 0), stop=(ks == NST - 1))
                                nc.tensor.matmul(
                                    av[:sub, 1, 0:DH + 1],
                                    lhsT=eAB[0:kw, ks, 1, qoff:qoff + sub],
                                    rhs=v_sb[0:kw, ks, 2 * hp + 1, :],
                                    start=(ks == 0), stop=(ks == NST - 1))
                            zrec = op.tile([P, 2], FP, tag="z", bufs=4)
                            nc.vector.reciprocal(zrec[:sub, 0:1],
                                                 av[:sub, 0, DH:DH + 1])
                            nc.vector.reciprocal(zrec[:sub, 1:2],
                                                 av[:sub, 1, DH:DH + 1])
                            o_sc = op.tile([P, P], BF, tag="osc", bufs=6)
                            nc.vector.tensor_scalar_mul(
                                o_sc[:sub, 0:DH], av[:sub, 0, 0:DH],
                                zrec[:sub, 0:1])
                            nc.vector.tensor_scalar_mul(
                                o_sc[:sub, DH:P], av[:sub, 1, 0:DH],
                                zrec[:sub, 1:2])
                            tp = av[:, 0, 192:256].bitcast(BF)
                            nc.tensor.transpose(tp[:, :sub], o_sc[:sub, :],
                                                ident[:sub, :sub])
                            nc.vector.tensor_copy(
                                out=ot_sb[:, hp, q0 + qoff:q0 + qoff + sub],
                                in_=tp[:, :sub])

            # ---- out = OT.T @ Wo + bo (weights prefetched during pair 6)
            with tc.tile_pool(name="outp", bufs=5) as outp:
                for st in range(NST):
                    sw = ST_SIZES[st]
                    s0 = st * P
                    for (c0, cw2) in [(0, 512), (512, 512)]:
                        psC = psum.tile([P, 512], FP, tag="pp", bufs=2)
                        for dt in range(KT):
                            nc.tensor.matmul(
                                psC[:sw, :cw2],
                                lhsT=ot_sb[:, dt, s0:s0 + sw],
                                rhs=wo_sb[:, dt, c0:c0 + cw2],
                                start=(dt == 0), stop=(dt == KT - 1))
                        o_sb = outp.tile([P, 512], FP)
                        nc.vector.tensor_add(o_sb[:sw, :cw2],
                                             psC[:sw, :cw2],
                                             bo_bc[:sw, c0:c0 + cw2])
                        nc.sync.dma_start(out=out[s0:s0 + sw, c0:c0 + cw2],
                                          in_=o_sb[:sw, :cw2])
    _legalize_syncs(nc)
    return nc


_NC_CACHE = []


def _get_nc():
    if not _NC_CACHE:
        _NC_CACHE.append(build_nc())
    return _NC_CACHE[0]


def _in_maps(x, Wq, bq, Wk, bk, Wv, bv, Wo, bo):
    bf = lambda a: np.ascontiguousarray(
        np.asarray(a, dtype=np.float32).astype(ml_dtypes.bfloat16))
    f = lambda a: np.ascontiguousarray(np.asarray(a, dtype=np.float32))
    shared = {"Wq": bf(Wq), "Wk": bf(Wk), "Wv": bf(Wv), "Wo": bf(Wo),
              "bq": f(bq), "bk": f(bk), "bv": f(bv), "bo": f(bo)}
    x = np.asarray(x, dtype=np.float32)
    return [{"xT": bf(x[b].T), **shared} for b in range(B)]


def kernel(x, Wq, bq, Wk, bk, Wv, bv, Wo, bo):
    nc = _get_nc()
    in_maps = _in_maps(x, Wq, bq, Wk, bk, Wv, bv, Wo, bo)
    res = run_bass_kernel_spmd(nc, in_maps, list(range(B)))
    return np.stack([res.results[b]["out"] for b in range(B)], axis=0)
